# revision 1
# baseline (speedup 1.0000x reference)
"""ActiveConv Trainium2 kernel, v8.

out[b,o,y,x] = sum_c conv_w[o,c] * bilinear_displace(repeat(inp,4)[b,c], offsets[c]) + conv_b[o]

Structure:
  * Host stages each displaced channel as a contiguous 65x66 bf16 window
    (integer shift + zero-pad baked into layout, pure memcpy); fractional
    x-corners are folded into 4 weight blocks; the fractional y-corner is a
    DVE pre-blend  gy = src[r] + (fy/(1-fy))*src[r+1]  with (1-fy) folded
    into the weights -> 4 matmul passes per 512-px tile.
  * 8 half-gathers ([128, 33*66] bf16, ~0.57 MB) on the sync HWDGE ring;
    blends at the same granularity (fewer DVE per-op overheads).
  * ~4us of warm-up matmuls on a zeroed tile flip the PE HAM throttle to
    2.4 GHz while the first gathers stream.
  * bf16 output (tolerance 2e-2; bf16 adds ~4e-3), halving output traffic.
  * bias-add PSUM->SBUF on scalar (vector for the last two tiles); outputs
    as 1024-px pairs on the scalar ring, last two tiles as singles split
    across both rings.
"""

import numpy as np
import ml_dtypes

B, C_IN, H, W = 16, 64, 64, 64
OPC = 4
C = C_IN * OPC          # 256
C_OUT = 128
NCORES = 8
BPC = B // NCORES       # batches per core
HW = H * W

WR, WC = 65, 66         # per-channel source window rows/cols
USR = 33                # src rows per gather/blend unit (32 output rows + 1)
FDU = USR * WC          # 2178 src elems per unit
FDBU = 32 * WC          # 2112 blended elems per unit

_PLAN_CACHE = {}


def _build_plan():
    import concourse.bacc as bacc
    import concourse.bass as bass
    import concourse.tile as tile
    import concourse.mybir as mybir

    nc = bacc.Bacc(None, target_bir_lowering=False)

    pbw = nc.dram_tensor("pbw", [BPC, C, WR * WC], mybir.dt.bfloat16, kind="ExternalInput")
    wts = nc.dram_tensor("wts", [128, 4 * 128], mybir.dt.bfloat16, kind="ExternalInput")
    fcon = nc.dram_tensor("fcon", [128, 3], mybir.dt.float32, kind="ExternalInput")
    out = nc.dram_tensor("out", [BPC, 128, HW], mybir.dt.bfloat16, kind="ExternalOutput")

    with tile.TileContext(nc) as tc:
        with (
            tc.tile_pool(name="const", bufs=1) as const,
            tc.tile_pool(name="graw", bufs=1) as graw,
            tc.tile_pool(name="gble", bufs=1) as gble,
            tc.tile_pool(name="psum", bufs=6, space="PSUM") as psum,
            tc.tile_pool(name="wps", bufs=1, space="PSUM") as wps,
            tc.tile_pool(name="outp", bufs=6) as outp,
        ):
            # consts first on the sync ring
            wts_t = const.tile([128, 4 * 128], mybir.dt.bfloat16)
            nc.sync.dma_start(out=wts_t[:], in_=wts[:])
            fcon_t = const.tile([128, 3], mybir.dt.float32)
            nc.sync.dma_start(out=fcon_t[:], in_=fcon[:])

            # HAM warm-up while gathers stream
            scr = const.tile([128, 512], mybir.dt.bfloat16)
            nc.gpsimd.memset(scr[:], 0)
            wpt = wps.tile([128, 512], mybir.dt.float32)
            for _ in range(10):
                nc.tensor.matmul(wpt[:], lhsT=scr[:, 0:128], rhs=scr[:],
                                 start=True, stop=True)

            raw = {}
            gy = {}
            for b in range(BPC):
                for u in range(2):
                    for h in range(2):
                        q = b * 2 + h
                        raw[q, u] = graw.tile([128, FDU], mybir.dt.bfloat16,
                                              name=f"r{q}_{u}")
                        gy[q, u] = gble.tile([128, FDBU], mybir.dt.bfloat16,
                                             name=f"g{q}_{u}")
            for b in range(BPC):
                for u in range(2):
                    for h in range(2):
                        q = b * 2 + h
                        nc.sync.dma_start(
                            out=raw[q, u][:],
                            in_=pbw[b, h * 128:(h + 1) * 128,
                                    u * 32 * WC: u * 32 * WC + FDU],
                        )

            # y-blend on DVE
            for b in range(BPC):
                for u in range(2):
                    for h in range(2):
                        q = b * 2 + h
                        rv = raw[q, u][:].rearrange("p (r w) -> p r w", w=WC)
                        nc.vector.tensor_scalar_mul(
                            gy[q, u][:],
                            rv[:, 1:USR, :],
                            fcon_t[:, h + 1:h + 2],
                        )
                        nc.vector.tensor_add(
                            gy[q, u][:].rearrange("p (r w) -> p r w", w=WC),
                            gy[q, u][:].rearrange("p (r w) -> p r w", w=WC),
                            rv[:, 0:32, :],
                        )

            ti = 0
            ot = None
            for b in range(BPC):
                for yt in range(8):
                    u, rbase = yt // 4, 8 * (yt % 4)
                    pt = psum.tile([128, 512], mybir.dt.float32)
                    j = 0
                    for h in range(2):
                        gv = gy[b * 2 + h, u][:].rearrange("p (r w) -> p r w", w=WC)
                        for t in range(2):
                            nc.tensor.matmul(
                                pt[:],
                                lhsT=wts_t[:, (h * 2 + t) * 128:(h * 2 + t + 1) * 128],
                                rhs=gv[:, rbase: rbase + 8, t: t + 64],
                                start=(j == 0), stop=(j == 3),
                            )
                            j += 1
                    last2 = (b == BPC - 1 and yt >= 6)
                    if yt % 2 == 0:
                        ot = outp.tile([128, 1024], mybir.dt.bfloat16)
                    osl = ot[:, (yt % 2) * 512:(yt % 2) * 512 + 512]
                    if last2:
                        nc.vector.tensor_scalar_add(osl, pt[:], fcon_t[:, 0:1])
                        eng = nc.sync if yt == 6 else nc.scalar
                        eng.dma_start(out=out[b, :, yt * 512:(yt + 1) * 512], in_=osl)
                    else:
                        nc.scalar.add(osl, pt[:], fcon_t[:, 0:1])
                        if yt % 2 == 1:
                            nc.scalar.dma_start(
                                out=out[b, :, (yt - 1) * 512:(yt + 1) * 512],
                                in_=ot[:],
                            )
                    ti += 1

    nc.finalize()
    return nc


def _prep(offsets, conv_w, conv_b):
    """Host-side folding of displacement into window layout + weights."""
    dx = offsets[:, 0].astype(np.float64)
    dy = offsets[:, 1].astype(np.float64)
    ix = np.floor(dx).astype(np.int64)
    iy = np.floor(dy).astype(np.int64)
    fx = (dx - ix).astype(np.float32)
    fy = (dy - iy).astype(np.float32)

    alive = (iy > -(H + 1)) & (iy < H) & (ix > -(W + 1)) & (ix < W)
    ix = np.where(alive, ix, 0)
    iy = np.where(alive, iy, 0)

    px0 = max(0, -int(ix.min()))
    px1 = max(0, int(ix.max()) + 2)
    py0 = max(0, -int(iy.min()))
    py1 = max(0, int(iy.max()) + 2)
    Hp, Wp = H + py0 + py1, W + px0 + px1

    one_m_fy = np.maximum(1.0 - fy, np.float32(1e-30)).astype(np.float32)
    alpha = (fy / one_m_fy).astype(np.float32)

    w = conv_w.astype(np.float32)
    wx = [(1.0 - fx), fx]
    wts = np.zeros((128, 4 * 128), dtype=np.float32)
    for h in range(2):
        cs = slice(h * 128, (h + 1) * 128)
        for t in range(2):
            m = (w[:, cs] * (wx[t][cs] * one_m_fy[cs] * alive[cs])[None, :])
            wts[:, (h * 2 + t) * 128:(h * 2 + t + 1) * 128] = m.T
    wts = wts.astype(ml_dtypes.bfloat16)

    fcon = np.stack([conv_b.astype(np.float32), alpha[:128], alpha[128:]],
                    axis=1)  # [128, 3]: bias | alpha_h0 | alpha_h1
    return dict(px0=px0, py0=py0, Hp=Hp, Wp=Wp, ix=ix, iy=iy,
                wts=wts, fcon=fcon)


def kernel(inp, offsets, conv_w, conv_b, _trace=False):
    import concourse.bass_utils as bu

    inp = np.asarray(inp)
    offsets = np.asarray(offsets)
    conv_w = np.asarray(conv_w)
    conv_b = np.asarray(conv_b)

    p = _prep(offsets, conv_w, conv_b)

    if "plan" not in _PLAN_CACHE:
        _PLAN_CACHE["plan"] = _build_plan()
    nc = _PLAN_CACHE["plan"]

    padded = np.zeros((B, C_IN, p["Hp"], p["Wp"]), dtype=ml_dtypes.bfloat16)
    padded[:, :, p["py0"]: p["py0"] + H, p["px0"]: p["px0"] + W] = inp.astype(
        ml_dtypes.bfloat16
    )
    cin = (np.arange(C) // OPC)[:, None, None]
    rows = (p["py0"] + p["iy"])[:, None, None] + np.arange(WR)[None, :, None]
    cols = (p["px0"] + p["ix"])[:, None, None] + np.arange(WC)[None, None, :]
    pbw = padded[:, cin, rows, cols]                     # [B, C, WR, WC]
    pbw = pbw.reshape(B, C, WR * WC)

    in_maps = []
    for core in range(NCORES):
        in_maps.append({
            "pbw": pbw[core * BPC:(core + 1) * BPC],
            "wts": p["wts"],
            "fcon": p["fcon"],
        })

    res = bu.run_bass_kernel_spmd(
        nc, in_maps, core_ids=list(range(NCORES)), trace=_trace
    )
    if _trace:
        kernel.last_exec_ns = res.exec_time_ns
        kernel.last_mean_exec_ns = res.mean_exec_time_ns
        it = res.instructions_and_trace
        kernel.last_trace_path = it[1] if it else None

    out = np.concatenate(
        [np.asarray(res.results[i]["out"]).astype(np.float32).reshape(BPC, C_OUT, H, W)
         for i in range(NCORES)],
        axis=0,
    )
    return out

